# revision 8
# baseline (speedup 1.0000x reference)
"""Trainium2 Bass kernel for nn_C4MoEVM (moe_routing).

Math: every softmax "lookup" in the reference is exactly one-hot in fp32
(scale=1000 => exp(-1000) underflows to 0), so the module reduces to:
  opcode 0: a+b        1: a-b       2: round(a*b) == a*b (exact, <=225)
  opcode 3,4,5: a&b, a|b, a^b  (integer bitwise on 4-bit values)
  opcode 6: y0 = fp32(1/z), z = 0.25 + temp/2, temp = b * 2^-e in [0.5,1),
            e = floor(log2 b)+1; two Newton steps y <- y*(2-temp*y);
            recip = y * 2^-e.
Routing gates are a numerically-exact one-hot selection by opcode (off-diag
gate leakage is ~2e-9 relative — negligible under a norm metric).

Strategy: pure data-parallel over 8 cores (B=262144 -> 32768/core laid out
as [128, 256]). Inputs are exact small integers, shipped as one packed int8
[128, 768] tensor per core (a|b|opcode). All compute is elementwise; the
recip expert uses fp32 exponent bit tricks (bitwise/shift ALU ops preserve
bits; arithmetic ALU ops upcast to fp32, so arithmetic only ever touches
small integers) and the RECIPROCAL_APPROX custom-DVE ops for 1/z and the
two Newton steps.

Engine note: DVE TensorTensor/CopyPredicated ISA slots hold at most ONE
embedded sync wait, so ops that would combine a cross-engine wait with a
same-engine RMW wait fail walrus codegen. The engine map keeps every
DVE op's wait count <= 1.
"""

import numpy as np

B = 262144
N_CORES = 8
PER_CORE = B // N_CORES  # 32768
P = 128
F = PER_CORE // P  # 256

_CACHE = {}

# engine for each op group; values: "vector" | "gpsimd"
ENGINES = {
    "conv": "vector",   # ia, ib, fbf converts
    "masks": "vector",  # opcode == k masks
    "tt": "vector",     # 6 expert TensorTensor ops
    "bits": "vector",   # exponent bit tricks
    "z": "vector",      # z = 0.25 + temp/2
    "fres": "vector",   # int16 -> fp32 convert of selected int expert
}


def _build_program():
    from contextlib import ExitStack

    import concourse.tile as tile
    from concourse import bacc, mybir
    from concourse.dve_ops import RECIPROCAL_APPROX_NR

    Alu = mybir.AluOpType
    dt = mybir.dt

    nc = bacc.Bacc("TRN2", target_bir_lowering=False, debug=False)
    abo8 = nc.declare_dram_parameter("abo8", [P, 3 * F], dt.int8, isOutput=False)
    out = nc.declare_dram_parameter("out", [P, F], dt.float32, isOutput=True)

    eng = {k: getattr(nc, v) for k, v in ENGINES.items()}

    with tile.TileContext(nc) as tc, ExitStack() as ctx:
        pool = ctx.enter_context(tc.tile_pool(name="p", bufs=1))

        def t(name, dtype, shape=(P, F)):
            return pool.tile(list(shape), dtype, name=name, tag=name)

        tin = t("tin", dt.int8, (P, 3 * F))
        nc.sync.dma_start(tin[:], abo8[:])
        a8 = tin[:, 0:F]
        b8 = tin[:, F : 2 * F]
        o8 = tin[:, 2 * F : 3 * F]

        # ---- converts ----
        ia = t("ia", dt.int16)
        eng["conv"].tensor_copy(ia[:], a8)
        ib = t("ib", dt.int16)
        eng["conv"].tensor_copy(ib[:], b8)
        fbf = t("fbf", dt.float32)
        eng["conv"].tensor_copy(fbf[:], b8)  # b as fp32

        # ---- masks (opcode == k) ----
        masks = []
        for k in range(1, 7):
            mk = t(f"m{k}", dt.uint8)
            eng["masks"].tensor_scalar(mk[:], o8, k, None, Alu.is_equal)
            masks.append(mk)

        # ---- exponent bit tricks (bitwise/shift ops only on large ints) ----
        fb = fbf[:].bitcast(dt.int32)
        btmp = t("btmp", dt.int32)  # bits of temp = b * 2^-e in [0.5, 1)
        eng["bits"].tensor_scalar(
            btmp[:], fb, 0x007FFFFF, 0x3F000000, Alu.bitwise_and, Alu.bitwise_or
        )
        nep = t("nep", dt.int32)  # ~(biased exponent of b) via shift+xor(-1)
        eng["bits"].tensor_scalar(
            nep[:], fb, 23, -1, Alu.logical_shift_right, Alu.bitwise_xor
        )
        s254 = t("s254", dt.int32)  # 253 - Eb = 127 - e  (small-int add: exact)
        eng["bits"].tensor_scalar(s254[:], nep[:], 254, None, Alu.add)
        pwi = t("pwi", dt.int32)  # bits of 2^-e
        eng["bits"].tensor_scalar(pwi[:], s254[:], 23, None, Alu.logical_shift_left)

        temp = btmp[:].bitcast(dt.float32)
        pw = pwi[:].bitcast(dt.float32)

        # ---- z = 0.25 + temp/2 (exact: equals the recip_x grid point) ----
        z = t("z", dt.float32)
        eng["z"].tensor_scalar(z[:], temp, 0.5, 0.25, Alu.mult, Alu.add)

        # ---- integer experts + predicated select ----
        ires = t("ires", dt.int16)
        eng["tt"].tensor_tensor(ires[:], ia[:], ib[:], Alu.add)
        isub = t("isub", dt.int16)
        eng["tt"].tensor_tensor(isub[:], ia[:], ib[:], Alu.subtract)
        imul = t("imul", dt.int16)
        eng["tt"].tensor_tensor(imul[:], ia[:], ib[:], Alu.mult)
        iand = t("iand", dt.int16)
        eng["tt"].tensor_tensor(iand[:], ia[:], ib[:], Alu.bitwise_and)
        ior = t("ior", dt.int16)
        eng["tt"].tensor_tensor(ior[:], ia[:], ib[:], Alu.bitwise_or)
        ixor = t("ixor", dt.int16)
        eng["tt"].tensor_tensor(ixor[:], ia[:], ib[:], Alu.bitwise_xor)

        for mk, val in zip(masks[:5], [isub, imul, iand, ior, ixor]):
            nc.vector.copy_predicated(ires[:], mk[:], val[:])

        fres = t("fres", dt.float32)
        eng["fres"].tensor_copy(fres[:], ires[:])

        # ---- reciprocal expert (custom-DVE ops) ----
        def nr(out_ap, x_ap, y_ap):
            # one Newton step: (2 - x*y) * y, single custom-DVE instruction
            nc.vector._custom_dve(
                RECIPROCAL_APPROX_NR, out=out_ap, in0=x_ap, in1=y_ap, s0=2.0
            )

        y0a = t("y0a", dt.float32)
        nc.vector.reciprocal_approx_fast(y0a[:], z[:])
        y0 = t("y0", dt.float32)
        nr(y0[:], z[:], y0a[:])  # y0 ~= fp32(1/z) (~2 ULP) == recip_val[idx]
        y1 = t("y1", dt.float32)
        nr(y1[:], temp, y0[:])
        y2 = t("y2", dt.float32)
        nr(y2[:], temp, y1[:])
        recipv = t("recipv", dt.float32)
        nc.vector.tensor_tensor(recipv[:], y2[:], pw, Alu.mult)

        nc.vector.copy_predicated(fres[:], masks[5][:], recipv[:])

        nc.sync.dma_start(out[:], fres[:])

    nc.compile()  # bacc legalization: event sems, nop fusion, reg alloc, ISA codegen
    return nc


def _get_program():
    if "nc" not in _CACHE:
        _CACHE["nc"] = _build_program()
    return _CACHE["nc"]


def _pack_inputs(a, b, opcode):
    """Shard + pack to one int8 [P, 3F] tensor per core."""
    a8 = a.astype(np.int8).reshape(N_CORES, P, F)
    b8 = b.astype(np.int8).reshape(N_CORES, P, F)
    o8 = opcode.astype(np.int8).reshape(N_CORES, P, F)
    return [
        np.ascontiguousarray(np.concatenate([a8[i], b8[i], o8[i]], axis=1))
        for i in range(N_CORES)
    ]


def run(a, b, opcode, trace=False):
    from concourse.bass_utils import run_bass_kernel_spmd

    nc = _get_program()
    in_maps = [{"abo8": m} for m in _pack_inputs(a, b, opcode)]
    res = run_bass_kernel_spmd(nc, in_maps, list(range(N_CORES)), trace=trace)
    out = np.concatenate([r["out"].reshape(-1) for r in res.results])
    return out.astype(np.float32, copy=False), res


def kernel(a, b, opcode, and_table, or_table, xor_table, recip_val):
    out, _ = run(np.asarray(a), np.asarray(b), np.asarray(opcode))
    return out
